# revision 9
# baseline (speedup 1.0000x reference)
"""Trainium2 Bass kernel for nn_DataEmbedding_ALLPE_Weighted.

Sharding: data-parallel over batch (16 batches / 8 cores = 2 per core).
All parameters replicated. No collectives.

Per-core pipeline (per batch, tokens-on-partitions [128 tok, 512 D] tiles):
  A. rolling stats via prefix sums / log-shift max chains in a
     [128 = 4 seg x 32 ch, 23+1024] folded layout; comb assembled in
     [256 cin, 4098] (channels-on-partitions) via SBUF->SBUF regroup DMAs.
  B. pe_lin = tape_pos @ M2.T + c1  (PE, once per core, SBUF-resident bf16)
  C. R0    = w2*LN_l-part + host-folded (w1*pf_ln + bias consts)  (once)
  D. main loop over 64 token tiles:
       conv (6 bf16 MMs + rank-1 bias) -> LN-c -> PE transpose ->
       folded mixer@tproj matmul + pe_lin -> LN-t ->
       out = PE(diag(w0 g_c) @ hcT + R0) + DVE fused (ht * w3 g_t)
"""

import numpy as np

import concourse.bass as bass
import concourse.mybir as mybir
import concourse.tile as tile
from concourse import bacc
from concourse.bass_utils import run_bass_kernel_spmd

F32 = mybir.dt.float32
BF16 = mybir.dt.bfloat16
AL = mybir.AluOpType
AF = mybir.ActivationFunctionType

B, L, C, D = 16, 4096, 32, 512
W = 24
LAGS = [3, 5, 7]
EPS = 1e-5
NCORES = 8
BPC = B // NCORES          # batches per core
NSEG = 4
SEG = L // NSEG            # 1024
PADL = W - 1               # 23
SEGP = SEG + PADL          # 1047
NT = L // 128              # 32 token tiles per batch
NTAP = 3


def _build_bass():
    nc = bacc.Bacc("TRN2", target_bir_lowering=False, debug=False,
                   num_devices=NCORES)

    def din(name, shape, dt=F32):
        return nc.dram_tensor(name, shape, dt, kind="ExternalInput").ap()

    t_xpad = din("xpad", [BPC, 128, SEGP])
    t_wr = din("wr", [2 * NTAP, 128, 512], BF16)     # idx = tap*2 + half
    t_convb = din("convb", [1, 512], BF16)
    t_ftg = din("ftg", [4, 128, 512], BF16)
    t_m2t = din("m2t", [4, 128, 512], BF16)
    t_c1 = din("c1", [1, 512], BF16)
    t_tapet = din("tapet", [4, 128, L], BF16)
    t_pel = din("pel", [NT, 128, 512])
    t_pfg = din("pfg", [NT, 128, 512])
    t_dg0 = din("dg0", [4, 128, 512], BF16)
    t_g3 = din("g3bc", [128, 512])
    t_gl = din("glbc", [128, 512])
    t_ident = din("ident", [128, 128], BF16)
    t_ones1 = din("ones1", [1, 128], BF16)
    t_out = nc.dram_tensor("out", [BPC, L, D], F32, kind="ExternalOutput").ap()

    with tile.TileContext(nc) as tc:
        _body(tc, nc, t_xpad, t_wr, t_convb, t_ftg, t_m2t, t_c1, t_tapet,
              t_pel, t_pfg, t_dg0, t_g3, t_gl, t_ident, t_ones1, t_out)
    nc.compile()
    return nc


def _body(tc, nc, t_xpad, t_wr, t_convb, t_ftg, t_m2t, t_c1, t_tapet,
          t_pel, t_pfg, t_dg0, t_g3, t_gl, t_ident, t_ones1, t_out):
    v = nc.vector
    gp = nc.gpsimd
    sc = nc.scalar
    pe = nc.tensor
    sy = nc.sync

    cpool = tc.alloc_tile_pool(name="consts", bufs=1)
    # resident constants
    wr_s = cpool.tile([128, 6 * 512], BF16)
    sy.dma_start(wr_s[:].rearrange("p (k n) -> p k n", k=6),
                 t_wr.rearrange("k p n -> p k n"))
    ftg_s = cpool.tile([128, 4 * 512], BF16)
    sy.dma_start(ftg_s[:].rearrange("p (k n) -> p k n", k=4),
                 t_ftg.rearrange("k p n -> p k n"))
    m2t_s = cpool.tile([128, 4 * 512], BF16)
    sy.dma_start(m2t_s[:].rearrange("p (k n) -> p k n", k=4),
                 t_m2t.rearrange("k p n -> p k n"))
    dg0_s = cpool.tile([128, 4 * 512], BF16)
    sy.dma_start(dg0_s[:].rearrange("p (k n) -> p k n", k=4),
                 t_dg0.rearrange("k p n -> p k n"))
    g3_s = cpool.tile([128, 512], F32)
    sy.dma_start(g3_s[:], t_g3)
    gl_s = cpool.tile([128, 512], F32)
    sy.dma_start(gl_s[:], t_gl)
    ident_s = cpool.tile([128, 128], BF16)
    sy.dma_start(ident_s[:], t_ident)
    ones1_s = cpool.tile([1, 128], BF16)
    sy.dma_start(ones1_s[:], t_ones1)
    convb_s = cpool.tile([1, 512], BF16)
    sy.dma_start(convb_s[:], t_convb)
    c1_s = cpool.tile([1, 512], BF16)
    sy.dma_start(c1_s[:], t_c1)
    zscan_s = cpool.tile([128, SEGP], F32)
    gp.memset(zscan_s[:], 0.0)
    eps_s = cpool.tile([128, 1], F32)
    gp.memset(eps_s[:], EPS)

    # big residents
    pelin_sb = cpool.tile([128, NT * 512], BF16)
    dpool = tc.alloc_tile_pool(name="dram", bufs=1, space="DRAM")
    r0_dr = dpool.tile([128, NT * 512], BF16)
    comb_sb = cpool.tile([128, 2 * 2 * (L + 2)], BF16)  # [b][half] blocks

    def comb_ap(b, half):
        off = (2 * b + half) * (L + 2)
        return comb_sb[:, off:off + L + 2]

    psum = tc.alloc_tile_pool(name="ps", bufs=2, space="PSUM")
    pa = tc.alloc_tile_pool(name="pa", bufs=1)
    pm = tc.alloc_tile_pool(name="pm", bufs=2)
    st = tc.alloc_tile_pool(name="st", bufs=4)

    # ---------------- Phase B: pe_lin = tape @ M2.T + c1 ----------------
    for i in range(NT):
        ps = psum.tile([128, 512], F32, tag="ppt")
        for j in range(4):
            ta = pm.tile([128, 128], BF16, tag="tapet")
            sy.dma_start(ta[:], t_tapet[j, :, 128 * i:128 * (i + 1)])
            pe.matmul(ps[:], ta[:], m2t_s[:, 512 * j:512 * (j + 1)],
                      start=(j == 0), stop=False)
        pe.matmul(ps[:], ones1_s[:], c1_s[:], start=False, stop=True)
        sc.copy(pelin_sb[:, 512 * i:512 * (i + 1)], ps[:])

    # ---------------- Phase C: R0 ----------------
    for i in range(NT):
        pl = pm.tile([128, 512], F32, tag="pl")
        sy.dma_start(pl[:], t_pel[i])
        pf = pm.tile([128, 512], F32, tag="pf")
        sy.dma_start(pf[:], t_pfg[i])
        sq = pm.tile([128, 512], F32, tag="sq")
        s1 = st.tile([128, 1], F32, tag="s1")
        s2 = st.tile([128, 1], F32, tag="s2")
        sc.activation(sq[:], pl[:], AF.Square, accum_out=s2[:])
        v.tensor_reduce(s1[:], pl[:], axis=mybir.AxisListType.X, op=AL.add)
        alpha = st.tile([128, 1], F32, tag="al")
        beta = st.tile([128, 1], F32, tag="be")
        _ln_smalls(nc, st, s1, s2, alpha, beta, eps_s)
        hl = pm.tile([128, 512], F32, tag="hl")
        sc.activation(hl[:], pl[:], AF.Identity, bias=beta[:], scale=alpha[:])
        t1 = pm.tile([128, 512], F32, tag="vA")
        v.tensor_tensor(t1[:], hl[:], gl_s[:], op=AL.mult)
        r0t = pm.tile([128, 512], BF16, tag="r0w")
        v.tensor_tensor(r0t[:], t1[:], pf[:], op=AL.add)
        sy.dma_start(r0_dr[:, 512 * i:512 * (i + 1)], r0t[:])

    # ---------------- Phase A: stats -> comb (per batch) ----------------
    for b in range(BPC):
        xp = pa.tile([128, SEGP], F32, tag="xp")
        sy.dma_start(xp[:], t_xpad[b])
        xsq = pa.tile([128, SEGP], F32, tag="xsq")
        sc.square(xsq[:], xp[:])
        cs = pa.tile([128, SEGP + 1], F32, tag="cs")
        gp.memset(cs[:, 0:1], 0.0)
        v.tensor_tensor_scan(cs[:, 1:SEGP + 1], zscan_s[:], xp[:], 0.0,
                             op0=AL.add, op1=AL.add)
        cs2 = pa.tile([128, SEGP + 1], F32, tag="cs2")
        gp.memset(cs2[:, 0:1], 0.0)
        v.tensor_tensor_scan(cs2[:, 1:SEGP + 1], zscan_s[:], xsq[:], 0.0,
                             op0=AL.add, op1=AL.add)
        wsum = pa.tile([128, SEG], BF16, tag="wsum")
        v.tensor_sub(wsum[:], cs[:, W:SEGP + 1], cs[:, 0:SEG])
        ssqw = pa.tile([128, SEG], F32, tag="ssqw")
        v.tensor_sub(ssqw[:], cs2[:, W:SEGP + 1], cs2[:, 0:SEG])
        wsq = pa.tile([128, SEG], F32, tag="wsq")
        v.tensor_mul(wsq[:], wsum[:], wsum[:])
        var23 = pa.tile([128, SEG], F32, tag="var23")
        v.scalar_tensor_tensor(var23[:], wsq[:], -1.0 / W, ssqw[:],
                               op0=AL.mult, op1=AL.add)
        v.tensor_scalar_max(var23[:], var23[:], 0.0)
        stdt = pa.tile([128, SEG], BF16, tag="stdt")
        sc.activation(stdt[:], var23[:], AF.Sqrt, scale=1.0 / (W - 1))
        # max chain (gpsimd) / min chain (vector)
        outs = {}
        for name, op, eng in (("mx", AL.max, v), ("mn", AL.min, v)):
            m2 = pa.tile([128, SEGP - 1], F32, tag=name + "2")
            eng.tensor_tensor(m2[:], xp[:, 0:SEGP - 1], xp[:, 1:SEGP], op=op)
            m4 = pa.tile([128, SEGP - 3], F32, tag=name + "4")
            eng.tensor_tensor(m4[:], m2[:, 0:SEGP - 3], m2[:, 2:SEGP - 1], op=op)
            m8 = pa.tile([128, SEGP - 7], F32, tag=name + "8")
            eng.tensor_tensor(m8[:], m4[:, 0:SEGP - 7], m4[:, 4:SEGP - 3], op=op)
            m16 = pa.tile([128, SEGP - 15], F32, tag=name + "16")
            eng.tensor_tensor(m16[:], m8[:, 0:SEGP - 15], m8[:, 8:SEGP - 7],
                              op=op)
            mo = pa.tile([128, SEG], BF16, tag=name + "o")
            eng.tensor_tensor(mo[:], m16[:, 8:SEG + 8], m8[:, 0:SEG], op=op)
            outs[name] = mo
        lags = []
        for l in LAGS:
            lg = pa.tile([128, SEG], BF16, tag=f"lag{l}")
            v.tensor_sub(lg[:], xp[:, PADL:SEGP], xp[:, PADL - l:SEGP - l])
            lags.append(lg)
        xbf = pa.tile([128, SEG], BF16, tag="xbf")
        v.tensor_copy(xbf[:], xp[:, PADL:SEGP])

        feats0 = [xbf, wsum, outs["mx"], outs["mn"]]
        feats1 = [stdt] + lags
        for half, feats in ((0, feats0), (1, feats1)):
            dst = comb_ap(b, half)
            for r, ft in enumerate(feats):
                for s in range(NSEG):
                    sy.dma_start(
                        dst[32 * r:32 * (r + 1), 1 + SEG * s:1 + SEG * (s + 1)],
                        ft[32 * s:32 * (s + 1), :])
        for half in (0, 1):
            dst = comb_ap(b, half)
            v.tensor_copy(dst[:, 0:1], dst[:, L:L + 1])
            v.tensor_copy(dst[:, L + 1:L + 2], dst[:, 1:2])

    # ---------------- Phase D: main loop ----------------
    for b in range(BPC):
        for i in range(NT):
            # conv
            ph = psum.tile([128, 512], F32, tag="ph")
            k = 0
            for tap in range(NTAP):
                for half in (0, 1):
                    lhsT = comb_ap(b, half)[:, 128 * i + tap:128 * i + tap + 128]
                    pe.matmul(ph[:], lhsT,
                              wr_s[:, 512 * (tap * 2 + half):
                                   512 * (tap * 2 + half) + 512],
                              start=(k == 0), stop=False)
                    k += 1
            pe.matmul(ph[:], ones1_s[:], convb_s[:], start=False, stop=True)
            # LN-c stats
            sqc = pm.tile([128, 512], F32, tag="sq")
            s2c = st.tile([128, 1], F32, tag="s2")
            sc.activation(sqc[:], ph[:], AF.Square, accum_out=s2c[:])
            s1c = st.tile([128, 1], F32, tag="s1")
            v.tensor_reduce(s1c[:], ph[:], axis=mybir.AxisListType.X, op=AL.add)
            al_c = st.tile([128, 1], F32, tag="al")
            be_c = st.tile([128, 1], F32, tag="be")
            _ln_smalls(nc, st, s1c, s2c, al_c, be_c, eps_s)
            hc = pm.tile([128, 512], BF16, tag="hc")
            sc.activation(hc[:], ph[:], AF.Identity, bias=be_c[:],
                          scale=al_c[:])
            # transpose
            pT = psum.tile([128, 512], BF16, tag="pT")
            for j in range(4):
                pe.transpose(pT[:, 128 * j:128 * (j + 1)],
                             hc[:, 128 * j:128 * (j + 1)], ident_s[:])
            hcT = pm.tile([128, 512], BF16, tag="hcT")
            sc.copy(hcT[:], pT[:])
            # mm2: pt = hc @ F_g.T + pe_lin(+c1)
            ppt = psum.tile([128, 512], F32, tag="ppt")
            for j in range(4):
                pe.matmul(ppt[:], hcT[:, 128 * j:128 * (j + 1)],
                          ftg_s[:, 512 * j:512 * (j + 1)],
                          start=(j == 0), stop=False)
            pe.matmul(ppt[:], ident_s[:], pelin_sb[:, 512 * i:512 * (i + 1)],
                      start=False, stop=True)
            # LN-t stats
            sqt = pm.tile([128, 512], F32, tag="sq")
            s2t = st.tile([128, 1], F32, tag="s2t")
            sc.activation(sqt[:], ppt[:], AF.Square, accum_out=s2t[:])
            s1t = st.tile([128, 1], F32, tag="s1t")
            v.tensor_reduce(s1t[:], ppt[:], axis=mybir.AxisListType.X,
                            op=AL.add)
            al_t = st.tile([128, 1], F32, tag="alt")
            be_t = st.tile([128, 1], F32, tag="bet")
            _ln_smalls(nc, st, s1t, s2t, al_t, be_t, eps_s)
            # out accumulation on PE
            po = psum.tile([128, 512], F32, tag="po")
            for j in range(4):
                pe.matmul(po[:], hcT[:, 128 * j:128 * (j + 1)],
                          dg0_s[:, 512 * j:512 * (j + 1)],
                          start=(j == 0), stop=False)
            r0r = pm.tile([128, 512], BF16, tag="r0r")
            sy.dma_start(r0r[:], r0_dr[:, 512 * i:512 * (i + 1)])
            pe.matmul(po[:], ident_s[:], r0r[:], start=False, stop=True)
            # v-terms + final
            vA = pm.tile([128, 512], F32, tag="vA")
            v.scalar_tensor_tensor(vA[:], ppt[:], al_t[:], g3_s[:],
                                   op0=AL.mult, op1=AL.mult)
            vB = pm.tile([128, 512], F32, tag="vB")
            v.scalar_tensor_tensor(vB[:], g3_s[:], be_t[:], po[:],
                                   op0=AL.mult, op1=AL.add)
            osb = pm.tile([128, 512], F32, tag="osb")
            v.tensor_tensor(osb[:], vA[:], vB[:], op=AL.add)
            sy.dma_start(t_out[b, 128 * i:128 * (i + 1), :], osb[:])

    for p in (st, pm, pa, psum, dpool, cpool):
        p.release()


def _ln_smalls(nc, st, s1, s2, alpha, beta, eps_s):
    """From s1=sum(h), s2=sum(h^2) over D=512 -> alpha=1/sqrt(var+eps),
    beta=-mean*alpha."""
    v = nc.vector
    sc = nc.scalar
    m = st.tile([128, 1], F32, tag="m")
    v.tensor_scalar_mul(m[:], s1[:], 1.0 / D)
    msq = st.tile([128, 1], F32, tag="msq")
    v.tensor_mul(msq[:], m[:], m[:])
    var = st.tile([128, 1], F32, tag="var")
    v.scalar_tensor_tensor(var[:], s2[:], 1.0 / D, msq[:],
                           op0=AL.mult, op1=AL.subtract)
    se = st.tile([128, 1], F32, tag="se")
    sc.activation(se[:], var[:], AF.Sqrt, bias=eps_s[:])
    v.reciprocal(alpha[:], se[:])
    v.scalar_tensor_tensor(beta[:], m[:], -1.0, alpha[:],
                           op0=AL.mult, op1=AL.mult)


_NC_CACHE = None


def _get_nc():
    global _NC_CACHE
    if _NC_CACHE is None:
        _NC_CACHE = _build_bass()
    return _NC_CACHE


def _host_prep(inputs):
    f32 = np.float32
    x = np.asarray(inputs["x"], f32)
    conv_w = np.asarray(inputs["conv_w"], f32)
    conv_b = np.asarray(inputs["conv_b"], f32)
    learned_pe = np.asarray(inputs["learned_pe"], f32)
    tape_pos = np.asarray(inputs["tape_pos"], f32)
    tproj_w = np.asarray(inputs["tproj_w"], f32)
    tproj_b = np.asarray(inputs["tproj_b"], f32)
    mixer_w = np.asarray(inputs["mixer_w"], f32)
    mixer_b = np.asarray(inputs["mixer_b"], f32)
    g_c, b_c = np.asarray(inputs["g_c"], f32), np.asarray(inputs["b_c"], f32)
    g_f, b_f = np.asarray(inputs["g_f"], f32), np.asarray(inputs["b_f"], f32)
    g_l, b_l = np.asarray(inputs["g_l"], f32), np.asarray(inputs["b_l"], f32)
    g_t, b_t = np.asarray(inputs["g_t"], f32), np.asarray(inputs["b_t"], f32)
    wp = np.asarray(inputs["weight_params"], f32)

    e = np.exp(wp - wp.max())
    w = (e / e.sum()).astype(f32)

    # xpad: [B, 4seg*32c, 1047]
    xpadL = np.pad(x, ((0, 0), (PADL, 0), (0, 0)), mode="edge")
    xT = np.ascontiguousarray(xpadL.transpose(0, 2, 1))       # [B, C, 4119]
    xpad = np.empty((B, NSEG, C, SEGP), f32)
    for s in range(NSEG):
        xpad[:, s] = xT[:, :, SEG * s:SEG * s + SEGP]
    xpad = xpad.reshape(B, 128, SEGP)

    # conv weights: fold mean scale, rearrange to [tap*2+half, 128, 512] bf16
    cw = conv_w.copy()
    cw[:, C:2 * C, :] /= W
    wr = np.empty((6, 128, 512), f32)
    for tap in range(NTAP):
        for half in (0, 1):
            wr[tap * 2 + half] = cw[:, 128 * half:128 * (half + 1), tap].T
    bf = np.dtype("bfloat16") if False else None

    def to_bf16(a):
        import ml_dtypes
        return a.astype(ml_dtypes.bfloat16)

    M1 = mixer_w[:, :D]
    M2 = mixer_w[:, D:]
    F = M1 @ tproj_w
    F_g = F * g_c[None, :]
    c1 = F @ b_c + M1 @ tproj_b + mixer_b
    ftg = np.ascontiguousarray(F_g.T).reshape(4, 128, 512)
    m2t = np.ascontiguousarray(M2.T).reshape(4, 128, 512)
    tapet = np.ascontiguousarray(tape_pos[:L].T).reshape(4, 128, L)

    # fixed sinusoidal PE table, normalized (pure constant)
    pos = np.arange(L, dtype=np.float64)
    div = np.exp(np.arange(0, D, 2, dtype=np.float64) * (-np.log(10000.0) / D))
    ang = pos[:, None] * div[None, :]
    tab = np.zeros((L, D), np.float64)
    tab[:, 0::2] = np.sin(ang)
    tab[:, 1::2] = np.cos(ang)
    tab = tab.astype(f32)
    mtab = tab.mean(-1, keepdims=True)
    vtab = ((tab - mtab) ** 2).mean(-1, keepdims=True)
    tabn = (tab - mtab) / np.sqrt(vtab + EPS)
    pfg = (w[1] * (g_f[None, :] * tabn + b_f[None, :])
           + (w[0] * b_c + w[3] * b_t + w[2] * b_l)[None, :]).astype(f32)

    dg0 = np.zeros((4, 128, 512), f32)
    for j in range(4):
        for r in range(128):
            dg0[j, r, 128 * j + r] = w[0] * g_c[128 * j + r]

    base = {
        "wr": to_bf16(wr),
        "convb": to_bf16(conv_b[None, :]),
        "ftg": to_bf16(ftg),
        "m2t": to_bf16(m2t),
        "c1": to_bf16(c1[None, :]),
        "tapet": to_bf16(tapet),
        "pel": learned_pe[0, :L].reshape(NT, 128, 512).copy(),
        "pfg": pfg.reshape(NT, 128, 512).copy(),
        "dg0": to_bf16(dg0),
        "g3bc": np.tile((w[3] * g_t)[None, :], (128, 1)).astype(f32),
        "glbc": np.tile((w[2] * g_l)[None, :], (128, 1)).astype(f32),
        "ident": to_bf16(np.eye(128, dtype=f32)),
        "ones1": to_bf16(np.ones((1, 128), f32)),
    }
    in_maps = []
    for c in range(NCORES):
        m = dict(base)
        m["xpad"] = np.ascontiguousarray(xpad[BPC * c:BPC * (c + 1)])
        in_maps.append(m)
    return in_maps


def kernel(**inputs):
    in_maps = _host_prep(inputs)
    nc = _get_nc()
    res = run_bass_kernel_spmd(nc, in_maps, core_ids=list(range(NCORES)))
    out = np.concatenate([r["out"] for r in res.results], axis=0)
    return out.astype(np.float32)
